# revision 1
# baseline (speedup 1.0000x reference)
"""Trainium2 Bass kernel for nn_Embed_38766374814290 (embedding_lookup).

Math: out[i,j,l,e] = A[m][e] + delta_s[i,j,l] * B[m][e]
  where m = (j < traj_len[i]), delta_s = where(m, mat2[traj_loc-1], 0),
  A[m] = emb_sl_w[m] + emb_tl_w[m],
  B[m] = (emb_su_w[m]-emb_sl_w[m])/SU + (emb_tu_w[m]-emb_tl_w[m])/TU.

Sharding: pure data parallel over batch N = 32 -> 4 rows per core x 8 cores.

Per-core kernel, per batch row i (128 positions):
  1. indirect-gather G[pos, l] = mat2x[idx[pos], l] in bf16 hi/lo halves
     (idx redirects invalid positions to an appended all-zero row 4096).
  2. For each 32-wide l-group: three PE transposes pack [Ghi; Glo; Ghi]
     l-slices into one [96, 128] PSUM tile (base partitions 0/32/64);
     one DVE copy evicts it to SBUF bf16 (rows 0-95 of the lhsT tile);
     a tiny DMA writes 4 constant rows [m, m, 1, 1] (rows 96-99).
  3. Four K=100 bf16 matmuls per l-group against constant block-diagonal
     rhs build out[pos, (l,e)] = G*B1 + m*dA + A0 = G*B1 + A[m] in one
     pass (three-term hi/lo products keep rel err ~1e-5).
  4. PSUM -> SBUF evictions are pure copies, split between DVE and ACT.
  5. Output rows DMA out with fully contiguous 32KB-per-partition
     descriptors (1 MiB per DMA).
"""
import os
import numpy as np
from contextlib import ExitStack

SU, TU = 10000.0, 86400.0
N, M, L, E = 32, 128, 128, 64
NLOC = 4096
NCORES = 8
ROWS = N // NCORES  # 4 batch rows per core

_CACHE = {}


def _install_profhook():
    """Optional: shim the missing antenv.axon_hooks so trace=True works."""
    import sys
    import types
    if "antenv.axon_hooks" in sys.modules:
        return True
    try:
        from trn_agent_boot.trn_boot import _ntff_profile_via_ctypes
    except Exception:
        return False
    hook = [None]
    mod = types.ModuleType("antenv.axon_hooks")
    mod.set_axon_ntff_profile_hook = lambda h: hook.__setitem__(0, h)
    mod.get_axon_ntff_profile_hook = lambda: hook[0]
    sys.modules["antenv.axon_hooks"] = mod
    try:
        mod.set_axon_ntff_profile_hook(
            _ntff_profile_via_ctypes("/opt/axon/libaxon_pjrt.so"))
    except Exception:
        return False
    return True


def _build():
    import concourse.bass as bass
    import concourse.tile as tile
    from concourse import bacc, mybir

    F32 = mybir.dt.float32
    BF16 = mybir.dt.bfloat16
    I32 = mybir.dt.int32

    nc = bacc.Bacc("TRN2", target_bir_lowering=False, debug=False,
                   enable_asserts=True, num_devices=NCORES)
    m2hi_d = nc.dram_tensor("m2hi", [NLOC + 1, L], BF16,
                            kind="ExternalInput").ap()
    m2lo_d = nc.dram_tensor("m2lo", [NLOC + 1, L], BF16,
                            kind="ExternalInput").ap()
    idx_d = nc.dram_tensor("idx", [ROWS, M], I32, kind="ExternalInput").ap()
    mrow_d = nc.dram_tensor("mrow", [ROWS, 4, 4 * M], BF16,
                            kind="ExternalInput").ap()
    rhs_d = nc.dram_tensor("rhs", [4, 100, 8 * E], BF16,
                           kind="ExternalInput").ap()
    ident_d = nc.dram_tensor("ident", [128, 128], BF16,
                             kind="ExternalInput").ap()
    out_d = nc.dram_tensor("out", [ROWS, M, L * E], F32,
                           kind="ExternalOutput").ap()

    with tile.TileContext(nc) as tc, ExitStack() as ctx:
        const = ctx.enter_context(tc.tile_pool(name="const", bufs=1))
        ipool = ctx.enter_context(tc.tile_pool(name="idxp", bufs=2))
        gpool = ctx.enter_context(tc.tile_pool(name="gath", bufs=2))
        gtpool = ctx.enter_context(tc.tile_pool(name="gt", bufs=4))
        opool = ctx.enter_context(tc.tile_pool(name="orow", bufs=3))
        pst = ctx.enter_context(tc.tile_pool(name="pst", bufs=2, space="PSUM"))
        pso = ctx.enter_context(tc.tile_pool(name="pso", bufs=6, space="PSUM"))

        ident = const.tile([128, 128], BF16)
        nc.sync.dma_start(ident[:], ident_d[:])
        # HAM warmup: ~3.5us of back-to-back matmuls at t=0 lifts the PE
        # clock gate to 8/8 before the real burst; store-paced gaps later
        # are too short for it to re-throttle. Results are never read.
        wrhs = const.tile([128, 8 * E], BF16)
        nc.vector.memset(wrhs[:], 0.0)
        wpo = pso.tile([128, 8 * E], F32, tag="po")
        for _ in range(20):
            nc.tensor.matmul(wpo[:], lhsT=ident[:], rhs=wrhs[:],
                             start=True, stop=True)
        rhs_tiles = []
        for s in range(4):
            rt = const.tile([100, 8 * E], BF16, tag=f"rhs{s}")
            nc.sync.dma_start(rt[:], rhs_d[s])
            rhs_tiles.append(rt)

        for i in range(ROWS):
            it = ipool.tile([128, 1], I32)
            nc.scalar.dma_start(it[:], idx_d[i, :, None])
            ghi = gpool.tile([128, L], BF16, tag="ghi")
            nc.gpsimd.indirect_dma_start(
                out=ghi[:], out_offset=None, in_=m2hi_d[:],
                in_offset=bass.IndirectOffsetOnAxis(ap=it[:, :1], axis=0))
            glo = gpool.tile([128, L], BF16, tag="glo")
            nc.gpsimd.indirect_dma_start(
                out=glo[:], out_offset=None, in_=m2lo_d[:],
                in_offset=bass.IndirectOffsetOnAxis(ap=it[:, :1], axis=0))
            orow = opool.tile([128, L * E], F32)
            gtrow = gtpool.tile([100, 4 * 128], BF16)
            nc.scalar.dma_start(gtrow[96:100, :], mrow_d[i])
            for gi in range(4):
                sl = slice(32 * gi, 32 * (gi + 1))
                gsl = slice(128 * gi, 128 * (gi + 1))
                pt = pst.tile([96, 128], BF16)
                nc.tensor.transpose(out=pt[0:32, :], in_=ghi[:, sl],
                                    identity=ident[:])
                nc.tensor.transpose(out=pt[32:64, :], in_=glo[:, sl],
                                    identity=ident[:])
                nc.tensor.transpose(out=pt[64:96, :], in_=ghi[:, sl],
                                    identity=ident[:])
                nc.vector.tensor_copy(out=gtrow[0:96, gsl], in_=pt[:])
                pos = []
                for s in range(4):
                    po = pso.tile([128, 8 * E], F32, tag="po")
                    nc.tensor.matmul(po[:], lhsT=gtrow[:, gsl],
                                     rhs=rhs_tiles[s][:],
                                     start=True, stop=True)
                    pos.append(po)
                for s in range(4):
                    win = 2048 * gi + 512 * s
                    dst = orow[:, win:win + 512]
                    if s < 2:
                        nc.vector.tensor_copy(out=dst, in_=pos[s][:])
                    else:
                        nc.scalar.copy(out=dst, in_=pos[s][:])
                nc.sync.dma_start(out_d[i][:, 2048 * gi:2048 * (gi + 1)],
                                  orow[:, 2048 * gi:2048 * (gi + 1)])
    nc.compile()
    return nc


def kernel(traj_loc, mat2, vec, traj_len, l_max, emb_sl_w, emb_su_w,
           emb_tl_w, emb_tu_w):
    import ml_dtypes
    from concourse import bass_utils

    BF = ml_dtypes.bfloat16
    traj_loc = np.asarray(traj_loc).astype(np.int64)
    mat2 = np.ascontiguousarray(np.asarray(mat2, dtype=np.float32))
    traj_len = np.asarray(traj_len).astype(np.int64)
    esl = np.asarray(emb_sl_w, dtype=np.float32)
    esu = np.asarray(emb_su_w, dtype=np.float32)
    etl = np.asarray(emb_tl_w, dtype=np.float32)
    etu = np.asarray(emb_tu_w, dtype=np.float32)

    # host prep: constants
    A = esl + etl                                            # [2, E]
    B = (esu - esl) / np.float32(SU) + (etu - etl) / np.float32(TU)
    mask = (np.arange(M)[None, :] < traj_len[:, None])       # [N, M]
    idx_full = np.where(mask, traj_loc - 1, NLOC).astype(np.int32)

    # bf16 hi/lo splits
    def split(x):
        hi = x.astype(BF)
        lo = (x - hi.astype(np.float32)).astype(BF)
        return hi, lo

    mat2x = np.concatenate([mat2, np.zeros((1, L), np.float32)], axis=0)
    m2hi, m2lo = split(mat2x)
    b1hi, b1lo = split(B[1])
    dA = A[1] - A[0]
    dAhi, dAlo = split(dA)
    a0hi, a0lo = split(A[0])

    # rhs[s] is [100, 8E]: rows 0-31 pair with GThi (x b1hi), rows 32-63
    # with GTlo (x b1hi), rows 64-95 with GThi again (x b1lo); row
    # 32*t+8*s+lp selects l' = lp within the window and scales e-block lp.
    # Rows 96-99 pair with lhsT rows [m, m, 1, 1]: m*dAhi + m*dAlo +
    # A0hi + A0lo, replicated across all 8 e-blocks.
    rhs = np.zeros((4, 100, 8 * E), BF)
    for s in range(4):
        for lp in range(8):
            rhs[s, 8 * s + lp, E * lp:E * (lp + 1)] = b1hi
            rhs[s, 32 + 8 * s + lp, E * lp:E * (lp + 1)] = b1hi
            rhs[s, 64 + 8 * s + lp, E * lp:E * (lp + 1)] = b1lo
        rhs[s, 96, :] = np.tile(dAhi, 8)
        rhs[s, 97, :] = np.tile(dAlo, 8)
        rhs[s, 98, :] = np.tile(a0hi, 8)
        rhs[s, 99, :] = np.tile(a0lo, 8)
    ident = np.eye(128, dtype=np.float32).astype(BF)

    # mrow[i] = [m, m, 1, 1] rows for lhsT rows 96-99, tiled 4x along the
    # free dim so one DMA fills all four gt windows of a row's wide tile.
    mrow_full = np.empty((N, 4, 4 * M), BF)
    mbf4 = np.tile(mask.astype(BF), (1, 4))
    mrow_full[:, 0, :] = mbf4
    mrow_full[:, 1, :] = mbf4
    mrow_full[:, 2, :] = np.ones((1, 4 * M), BF)
    mrow_full[:, 3, :] = np.ones((1, 4 * M), BF)

    if "nc" not in _CACHE:
        _CACHE["nc"] = _build()
    nc = _CACHE["nc"]

    in_maps = []
    for c in range(NCORES):
        sl = slice(ROWS * c, ROWS * (c + 1))
        in_maps.append({
            "m2hi": m2hi,
            "m2lo": m2lo,
            "idx": np.ascontiguousarray(idx_full[sl]),
            "mrow": np.ascontiguousarray(mrow_full[sl]),
            "rhs": rhs,
            "ident": ident,
        })

    trace = os.environ.get("KERNEL_TRACE", "0") == "1" and _install_profhook()
    res = bass_utils.run_bass_kernel_spmd(
        nc, in_maps, core_ids=list(range(NCORES)), trace=bool(trace))
    if trace:
        _CACHE["exec_time_ns"] = res.exec_time_ns
        _CACHE["trace_path"] = (res.instructions_and_trace or (None, None))[1]
        _CACHE["tmpdir"] = res.profile_json

    out = np.concatenate(
        [res.results[c]["out"].reshape(ROWS, M, L, E) for c in range(NCORES)],
        axis=0)
    return out



# revision 10
# speedup vs baseline: 1.2449x; 1.2449x over previous
"""Trainium2 Bass kernel for nn_Embed_38766374814290 (embedding_lookup).

Math: out[i,j,l,e] = A[m][e] + delta_s[i,j,l] * B[m][e]
  where m = (j < traj_len[i]), delta_s = where(m, mat2[traj_loc-1], 0),
  A[m] = emb_sl_w[m] + emb_tl_w[m],
  B[m] = (emb_su_w[m]-emb_sl_w[m])/SU + (emb_tu_w[m]-emb_tl_w[m])/TU.

Sharding: pure data parallel over batch N = 32 -> 4 rows per core x 8 cores.

v5 design (bf16 output, rel-err gate is 2e-2 and bf16 adds ~2e-3):
  - Output tensor is bf16: halves the HBM write bytes (16MiB->8MiB/core);
    per-core aggregate DMA is ~400-420 GB/s regardless of queue count.
  - Gather table m2e[4097, 4, 36]: per l-group 32 mat2 bf16 columns plus
    const columns [m, m, 1, 1] baked in (the appended zero row carries
    m=0 but keeps the 1s), so ONE contiguous indirect DMA per batch row
    yields transpose-ready [128, 4, 36] tiles with mask handling free.
  - HAM keeps the PE at 1.2 GHz for this burst pattern (measured: the
    8/8 state never persists), so the kernel is designed to be fast at
    the COLD clock: l-groups are processed in PAIRS with tile_position
    row concurrency - the even l-group occupies PE rows 0-63, the odd
    one rows 64-127, and consecutive matmuls execute concurrently.
    No warmup block (it only delayed the pipeline by 5us).
  - Transposes are col-tiled into one [128,256] PSUM tile per pair
    (even -> partitions 0:36 free 0:128, odd -> 64:100 free 128:256),
    one DVE copy stages both lhsT blocks.
  - rhs duplicated at partition 64 so the odd-tile matmuls can read it.
  - K=36, [128,1024] double-bank PSUM tiles; f32->bf16 eviction split
    DVE/ACT (GPSIMD cannot access PSUM). Output DMA on the sync queue.
"""
import os
import numpy as np
from contextlib import ExitStack

SU, TU = 10000.0, 86400.0
N, M, L, E = 32, 128, 128, 64
NLOC = 4096
NCORES = 8
ROWS = N // NCORES  # 4 batch rows per core
K = 36  # 32 G^T rows + [m, m, 1, 1]

_CACHE = {}


def _install_profhook():
    """Optional: shim the missing antenv.axon_hooks so trace=True works."""
    import sys
    import types
    if "antenv.axon_hooks" in sys.modules:
        return True
    try:
        from trn_agent_boot.trn_boot import _ntff_profile_via_ctypes
    except Exception:
        return False
    hook = [None]
    mod = types.ModuleType("antenv.axon_hooks")
    mod.set_axon_ntff_profile_hook = lambda h: hook.__setitem__(0, h)
    mod.get_axon_ntff_profile_hook = lambda: hook[0]
    sys.modules["antenv.axon_hooks"] = mod
    try:
        mod.set_axon_ntff_profile_hook(
            _ntff_profile_via_ctypes("/opt/axon/libaxon_pjrt.so"))
    except Exception:
        return False
    return True


def _build():
    import concourse.bass as bass
    import concourse.tile as tile
    from concourse import bacc, mybir

    F32 = mybir.dt.float32
    BF16 = mybir.dt.bfloat16
    I32 = mybir.dt.int32

    nc = bacc.Bacc("TRN2", target_bir_lowering=False, debug=False,
                   enable_asserts=True, num_devices=NCORES)
    m2e_d = nc.dram_tensor("m2e", [NLOC + 1, 4 * K], BF16,
                           kind="ExternalInput").ap()
    idx_d = nc.dram_tensor("idx", [ROWS, M], I32, kind="ExternalInput").ap()
    rhs_d = nc.dram_tensor("rhs", [100, 4 * 512], BF16,
                           kind="ExternalInput").ap()
    ident_d = nc.dram_tensor("ident", [128, 128], BF16,
                             kind="ExternalInput").ap()
    out_d = nc.dram_tensor("out", [ROWS, M, L * E], BF16,
                           kind="ExternalOutput").ap()

    with tile.TileContext(nc) as tc, ExitStack() as ctx:
        const = ctx.enter_context(tc.tile_pool(name="const", bufs=1))
        ipool = ctx.enter_context(tc.tile_pool(name="idxp", bufs=ROWS))
        gpool = ctx.enter_context(tc.tile_pool(name="gath", bufs=ROWS))
        lpool = ctx.enter_context(tc.tile_pool(name="lhs", bufs=3))
        opool = ctx.enter_context(tc.tile_pool(name="orow", bufs=2))
        pst = ctx.enter_context(tc.tile_pool(name="pst", bufs=2, space="PSUM"))
        pso = ctx.enter_context(tc.tile_pool(name="pso", bufs=3, space="PSUM"))

        # idx first: the gathers depend on it; one [128,1] tile per batch
        # row (the indirect-DMA offset AP needs free offset 0)
        its = []
        for i in range(ROWS):
            it = ipool.tile([128, 1], I32)
            nc.sync.dma_start(it[:], idx_d[i, :, None])
            its.append(it)
        ident = const.tile([128, 128], BF16)
        nc.sync.dma_start(ident[:], ident_d[:])
        rhs = const.tile([100, 4 * 512], BF16, tag="rhs")
        nc.sync.dma_start(rhs[:], rhs_d[:])

        # all four gathers upfront (gpsimd SWDGE); each yields a full
        # transpose-ready [128, 4, 36] tile
        # NOTE: the indirect-DMA out AP must be 2-D ([128, 144]); a 3-D
        # tile view scatters the gathered rows incorrectly.
        g4s = []
        for i in range(ROWS):
            g4 = gpool.tile([128, 4 * K], BF16)
            nc.gpsimd.indirect_dma_start(
                out=g4[:], out_offset=None, in_=m2e_d[:],
                in_offset=bass.IndirectOffsetOnAxis(ap=its[i][:], axis=0))
            g4s.append(g4)

        def transpose_pair(i, q):
            """Transpose l-groups (2q, 2q+1) of row i into one lhsT tile:
            even block at partitions 0:36 free 0:128, odd at 64:100
            free 128:256 (= PE row strips 0-63 / 64-127)."""
            pt = pst.tile([128, 256], BF16)
            nc.tensor.transpose(out=pt[0:K, 0:128],
                                in_=g4s[i][:, 2 * q * K:(2 * q + 1) * K],
                                identity=ident[:])
            nc.tensor.transpose(out=pt[64:64 + K, 128:256],
                                in_=g4s[i][:, (2 * q + 1) * K:(2 * q + 2) * K],
                                identity=ident[:])
            lq = lpool.tile([128, 256], BF16)
            nc.vector.tensor_copy(out=lq[:], in_=pt[:])
            return lq

        # eviction rotation (32 evictions of [128,1024]): ACT slightly
        # more (DVE also stages the lhsT copies)
        vpos = {round(k * 32 / 14) for k in range(14)}
        ev_i = 0

        lq_next = transpose_pair(0, 0)
        for i in range(ROWS):
            orow = opool.tile([128, L * E], BF16)
            for q in range(2):
                lq = lq_next
                if not (i == ROWS - 1 and q == 1):
                    ni, nq = (i, 1) if q == 0 else (i + 1, 0)
                    lq_next = transpose_pair(ni, nq)
                # 4 concurrent-pair slots; [128,1024] tiles pair s in halves
                poE = [pso.tile([128, 1024], F32, tag="po", name=f"poE{h}")
                       for h in range(2)]
                poO = [pso.tile([128, 1024], F32, tag="po", name=f"poO{h}")
                       for h in range(2)]
                for s in range(4):
                    h, s2 = divmod(s, 2)
                    nc.tensor.matmul(
                        poE[h][:, 512 * s2:512 * (s2 + 1)],
                        lhsT=lq[0:K, 0:128],
                        rhs=rhs[0:K, 512 * s:512 * (s + 1)],
                        start=True, stop=True)
                    nc.tensor.matmul(
                        poO[h][:, 512 * s2:512 * (s2 + 1)],
                        lhsT=lq[64:64 + K, 128:256],
                        rhs=rhs[64:64 + K, 512 * s:512 * (s + 1)],
                        start=True, stop=True)
                for h in range(2):
                    for which, po in (("E", poE[h]), ("O", poO[h])):
                        lg = 2 * q if which == "E" else 2 * q + 1
                        dst = orow[:, 2048 * lg + 1024 * h:
                                   2048 * lg + 1024 * (h + 1)]
                        if ev_i in vpos:
                            nc.vector.tensor_copy(out=dst, in_=po[:])
                        else:
                            nc.scalar.copy(out=dst, in_=po[:])
                        ev_i += 1
                for lg in (2 * q, 2 * q + 1):
                    nc.sync.dma_start(
                        out_d[i][:, 2048 * lg:2048 * (lg + 1)],
                        orow[:, 2048 * lg:2048 * (lg + 1)])
    nc.compile()
    return nc


def kernel(traj_loc, mat2, vec, traj_len, l_max, emb_sl_w, emb_su_w,
           emb_tl_w, emb_tu_w):
    import ml_dtypes
    from concourse import bass_utils

    BF = ml_dtypes.bfloat16
    traj_loc = np.asarray(traj_loc).astype(np.int64)
    mat2 = np.ascontiguousarray(np.asarray(mat2, dtype=np.float32))
    traj_len = np.asarray(traj_len).astype(np.int64)
    esl = np.asarray(emb_sl_w, dtype=np.float32)
    esu = np.asarray(emb_su_w, dtype=np.float32)
    etl = np.asarray(emb_tl_w, dtype=np.float32)
    etu = np.asarray(emb_tu_w, dtype=np.float32)

    # host prep: constants
    A = esl + etl                                            # [2, E]
    B = (esu - esl) / np.float32(SU) + (etu - etl) / np.float32(TU)
    mask = (np.arange(M)[None, :] < traj_len[:, None])       # [N, M]
    idx_full = np.where(mask, traj_loc - 1, NLOC).astype(np.int32)

    def split(x):
        hi = x.astype(BF)
        lo = (x - hi.astype(np.float32)).astype(BF)
        return hi, lo

    b1 = B[1].astype(BF)
    dA = A[1] - A[0]
    dAhi, dAlo = split(dA)
    a0hi, a0lo = split(A[0])

    # gather table [4097, 4, 36]: per l-group 32 mat2 columns + [m, m, 1, 1].
    # Invalid positions index the appended row 4096: zeros + [0, 0, 1, 1].
    m2e = np.zeros((NLOC + 1, 4, K), BF)
    m2bf = mat2.astype(BF)
    for lg in range(4):
        m2e[:NLOC, lg, 0:32] = m2bf[:, 32 * lg:32 * (lg + 1)]
    m2e[:NLOC, :, 32] = 1
    m2e[:NLOC, :, 33] = 1
    m2e[:, :, 34] = 1
    m2e[:, :, 35] = 1

    # rhs [100, 4*512]: rows 0-35 = patterns (8s+lp scales e-block lp by B1,
    # rows 32-35 = dAhi/dAlo/A0hi/A0lo tiled); rows 64-99 = the same block
    # again so odd-tile matmuls (lhsT at partition 64) can read it.
    rhs = np.zeros((100, 4 * 512), BF)
    for s in range(4):
        for lp in range(8):
            rhs[8 * s + lp, 512 * s + E * lp:512 * s + E * (lp + 1)] = b1
        rhs[32, 512 * s:512 * (s + 1)] = np.tile(dAhi, 8)
        rhs[33, 512 * s:512 * (s + 1)] = np.tile(dAlo, 8)
        rhs[34, 512 * s:512 * (s + 1)] = np.tile(a0hi, 8)
        rhs[35, 512 * s:512 * (s + 1)] = np.tile(a0lo, 8)
    rhs[64:64 + K] = rhs[0:K]
    ident = np.eye(128, dtype=np.float32).astype(BF)

    if "nc" not in _CACHE:
        _CACHE["nc"] = _build()
    nc = _CACHE["nc"]

    in_maps = []
    for c in range(NCORES):
        sl = slice(ROWS * c, ROWS * (c + 1))
        in_maps.append({
            "m2e": m2e.reshape(NLOC + 1, 4 * K),
            "idx": np.ascontiguousarray(idx_full[sl]),
            "rhs": rhs,
            "ident": ident,
        })

    trace = os.environ.get("KERNEL_TRACE", "0") == "1" and _install_profhook()
    res = bass_utils.run_bass_kernel_spmd(
        nc, in_maps, core_ids=list(range(NCORES)), trace=bool(trace))
    if trace:
        _CACHE["exec_time_ns"] = res.exec_time_ns
        _CACHE["trace_path"] = (res.instructions_and_trace or (None, None))[1]
        _CACHE["tmpdir"] = res.profile_json

    out = np.concatenate(
        [res.results[c]["out"].reshape(ROWS, M, L, E) for c in range(NCORES)],
        axis=0).astype(np.float32)
    return out
